# revision 33
# baseline (speedup 1.0000x reference)
"""Self-contained 8-core Trainium2 Bass kernel for multi-head attention.

Problem: B=4, S=2048, E=1024, H=16, D=64 MHA with key-position mask.
Sharding: 8 cores = 4 batches x 2 head-groups (8 heads / 512 feats each).
Each core computes QKV projections for its (batch, head-group), attention,
and a partial output projection; the host sums the two partial outputs per
batch (bo folded into group-0 cores) in fp32.

Perf design (vs the v1 kernel):
  - Host pre-transposes + pre-slabs every tensor into the exact SBUF layout
    ([128, EC, S] etc.), so all device DMAs are full-rate plain copies --
    no on-device DMA transposes, and V-projection starts ~4us in.
  - The attention inner loop is software-pipelined per (pr, j, qt) unit:
    scores(kb+1) is emitted before attnV(kb), so the PE never sits waiting
    on the ACT exp; the ACT engine paces the loop at ~1.04us/kb.
  - All remaining projection work (K/Q proj of later head-pairs, out-proj)
    is chopped into ~1.7us matmul "quanta" and interleaved into the
    ACT-bound attention stream as PE filler.  The PE therefore runs
    near-continuously, which also keeps the HAM clock-gate at 2.4 GHz
    (idle-y kernels get throttled to 1.2 GHz).
  - PSUM budget (8 banks): scores double-buffer 2x[128,1024] (4 banks) +
    attnV accumulator [65,1024] (2) + proj filler [128,512] (1) +
    out-proj filler [128,512] (1).
  - mask folded into V: V_aug = [mask*V | mask] per head, so the softmax
    mask AND the denominator (row 64 of the attnV PSUM) are free; keys are
    host-compacted to the unmasked positions (exact), padded to s_k=1152.
  - exp has no max-subtraction (scores ~ N(0,1)); 1/sqrt(D) folded into
    the ACT scale field.  Output is bf16 (host accumulates in fp32).
"""

import os
import numpy as np

B = 4
S = 2048
E = 1024
H = 16
D = 64
G = 2                 # head groups (tensor parallel)
HL = H // G           # heads per core = 8
FL = HL * D           # local features = 512
P = 128
EC = E // P           # 8 e-chunks
FC = FL // P          # 4 local feature chunks
NCORES = 8
QW = 1024             # q tile width in attention

SK_PAD = 1152   # compacted key-dim (5.7 sigma above Binomial(2048, .5) mean)

_NC_CACHE = {}
LAST_RESULTS = None


def build_nc(s=S, s_k=SK_PAD, repeat=1, phases="ABCD"):
    """Build (and cache) the single-core Bass module, SPMD across 8 cores.

    repeat > 1 re-emits the whole computation N times inside one NEFF
    (timing aid: device time scales with repeat, launch overhead doesn't).
    """
    key = (s, s_k, repeat, phases)
    if key in _NC_CACHE:
        return _NC_CACHE[key]

    import concourse.mybir as mybir
    import concourse.tile as tile
    from concourse import bacc

    f32 = mybir.dt.float32
    bf16 = mybir.dt.bfloat16
    EXP = mybir.ActivationFunctionType.Exp

    n_kb = s_k // P       # key blocks
    n_qt = s // QW        # q units per (pr, j)
    n_qb = s // P         # out-proj q blocks
    kq_chunks = []        # (offset, width<=512) chunks covering s_k
    off = 0
    while off < s_k:
        w = min(512, s_k - off)
        kq_chunks.append((off, w))
        off += w
    # input DMA chunks for xk/xv: groups of 3 key-blocks (384 cols = 768B
    # contiguous per (p, ec) run -> full DMA rate, but fine-grained enough
    # that V-proj starts after the first chunk).
    x_chunks = []
    off = 0
    while off < s_k:
        w = min(3 * P, s_k - off)
        x_chunks.append((off, w))
        off += w
    # xv lands on the critical path to the first PE work: start with a
    # single key-block (half-rate 256B runs but tiny) then ramp up.
    v_chunks = [(0, P), (P, 2 * P)] + [(c0, cw) for c0, cw in x_chunks
                                       if c0 >= 3 * P]

    nc = bacc.Bacc("TRN2", target_bir_lowering=False, debug=False,
                   num_devices=NCORES)

    xq_d = nc.dram_tensor("xq", [P, EC, s], bf16, kind="ExternalInput").ap()
    xk_d = nc.dram_tensor("xk", [P, EC, s_k], bf16, kind="ExternalInput").ap()
    xv_d = nc.dram_tensor("xv", [P, EC, s_k], bf16, kind="ExternalInput").ap()
    wq_d = nc.dram_tensor("wq", [P, EC, FL], bf16, kind="ExternalInput").ap()
    wk_d = nc.dram_tensor("wk", [P, EC, FL], bf16, kind="ExternalInput").ap()
    wv_d = nc.dram_tensor("wv", [P, EC, FL], bf16, kind="ExternalInput").ap()
    wo_d = nc.dram_tensor("wo", [P, FC, E], bf16, kind="ExternalInput").ap()
    bq_d = nc.dram_tensor("bq", [P, FC], f32, kind="ExternalInput").ap()
    bk_d = nc.dram_tensor("bk", [P, FC], f32, kind="ExternalInput").ap()
    bv_d = nc.dram_tensor("bv", [FL], f32, kind="ExternalInput").ap()
    bo_d = nc.dram_tensor("bo", [E], f32, kind="ExternalInput").ap()
    mask_d = nc.dram_tensor("maskf", [s_k], f32, kind="ExternalInput").ap()
    out_d = nc.dram_tensor("out", [s, E], bf16, kind="ExternalOutput").ap()

    with tile.TileContext(nc) as tc:
      for _rep in range(repeat):
        with tc.tile_pool(name=f"consts{_rep}", bufs=1) as consts, \
             tc.tile_pool(name=f"persist{_rep}", bufs=1) as persist, \
             tc.tile_pool(name=f"xtkq{_rep}", bufs=1) as xtkq, \
             tc.tile_pool(name=f"ep{_rep}", bufs=4) as epool, \
             tc.tile_pool(name=f"np{_rep}", bufs=2) as npool, \
             tc.tile_pool(name=f"dout{_rep}", bufs=3) as dout:

            # ---------- constant / persistent SBUF ----------
            bv_row = consts.tile([1, FL], f32, tag="bv_row")
            maskc = consts.tile([P, n_kb], f32, tag="maskc")
            wv_sb = consts.tile([P, EC, FL], bf16, tag="wv")
            wk_sb = consts.tile([P, EC, FL], bf16, tag="wk")
            wq_sb = consts.tile([P, EC, FL], bf16, tag="wq")
            bk_sb = consts.tile([P, FC], f32, tag="bk")
            bq_sb = consts.tile([P, FC], f32, tag="bq")
            wo_sb = consts.tile([P, FC, E], bf16, tag="wo")
            bo_row = consts.tile([1, E], f32, tag="bo_row")

            xt_k = xtkq.tile([P, EC, s_k], bf16, tag="xtk")
            xt_q = xtkq.tile([P, EC, s], bf16, tag="xtq")

            KT = persist.tile([P, FC, s_k], bf16, tag="KT")
            QT = persist.tile([P, FC, s], bf16, tag="QT")
            AC = persist.tile([P, FC, s], bf16, tag="AC")     # attn_cat^T
            vaug = [persist.tile([P, HL * (D + 1)], bf16, tag=f"vaug{kb}",
                                 name=f"vaug{_rep}_{kb}")
                    for kb in range(n_kb)]

            # ---------- input DMAs ----------
            # All on the sync (SP) ring, in strict priority order (the V
            # path first so V-proj starts ~6us in).  Keeping the ACT ring
            # DMA-free leaves the Activation sequencer 100% for exps.
            with tc.tile_pool(name=f"xtv{_rep}", bufs=1) as xtvp:
                xt_v = xtvp.tile([P, EC, s_k], bf16, tag="xtv")
                nc.sync.dma_start(wv_sb[:, 0:EC // 2, :],
                                  wv_d[:, 0:EC // 2, :])
                c0f, cwf = v_chunks[0]
                nc.sync.dma_start(xt_v[:, :, c0f:c0f + cwf],
                                  xv_d[:, :, c0f:c0f + cwf])
                nc.sync.dma_start(wv_sb[:, EC // 2:, :],
                                  wv_d[:, EC // 2:, :])
                for c0, cw in v_chunks[1:]:
                    nc.sync.dma_start(xt_v[:, :, c0:c0 + cw],
                                      xv_d[:, :, c0:c0 + cw])
                nc.sync.dma_start(bv_row, bv_d[None, :])
                nc.sync.dma_start(maskc,
                                  mask_d.rearrange("(c p) -> p c", p=P))
                nc.sync.dma_start(wk_sb, wk_d)
                nc.sync.dma_start(bk_sb, bk_d)
                for c0, cw in x_chunks:
                    nc.sync.dma_start(xt_k[:, :, c0:c0 + cw],
                                      xk_d[:, :, c0:c0 + cw])
                nc.sync.dma_start(wq_sb, wq_d)
                nc.sync.dma_start(bq_sb, bq_d)
                for c0 in range(0, s, QW):
                    nc.sync.dma_start(xt_q[:, :, c0:c0 + QW],
                                      xq_d[:, :, c0:c0 + QW])
                nc.sync.dma_start(wo_sb, wo_d)
                nc.sync.dma_start(bo_row, bo_d[None, :])

                # broadcast rows (gpsimd)
                bv_bc = consts.tile([P, FL], f32, tag="bv_bc")
                nc.gpsimd.partition_broadcast(bv_bc, bv_row)
                bo_bc = consts.tile([P, E], f32, tag="bo_bc")
                nc.gpsimd.partition_broadcast(bo_bc, bo_row)
                maskc_bf = consts.tile([P, n_kb], bf16, tag="maskc_bf")
                nc.vector.tensor_copy(maskc_bf, maskc)

                # ---------- head phase: V projection + pr0 K/Q ----------
                with tc.tile_pool(name=f"vps{_rep}", bufs=2,
                                  space="PSUM") as vps:
                    for kb in range(n_kb):
                        ps = vps.tile([P, FL], f32, tag="vps", name="ps_v")
                        for ec in range(EC):
                            nc.tensor.matmul(
                                ps, lhsT=xt_v[:, ec, kb * P:(kb + 1) * P],
                                rhs=wv_sb[:, ec, :],
                                start=(ec == 0), stop=(ec == EC - 1))
                        vrow = vaug[kb].rearrange("p (h c) -> p h c", c=D + 1)
                        nc.vector.tensor_add(
                            vrow[:, :, 0:D],
                            ps.rearrange("p (h d) -> p h d", d=D),
                            bv_bc.rearrange("p (h d) -> p h d", d=D))
                        nc.vector.tensor_scalar_mul(
                            vrow[:, :, 0:D], vrow[:, :, 0:D],
                            maskc[:, kb:kb + 1])
                        nc.vector.tensor_copy(
                            vrow[:, :, D:D + 1],
                            maskc_bf[:, kb:kb + 1, None].to_broadcast(
                                [P, HL, 1]))

                    def kq_chunk(psum_pool, kind, pr, c0, cw):
                        w_sb, b_sb, xt, OUT = (
                            (wk_sb, bk_sb, xt_k, KT) if kind == "k"
                            else (wq_sb, bq_sb, xt_q, QT))
                        ps = psum_pool.tile([P, 512], f32, tag="kq",
                                            name=f"ps_{kind}")
                        for ec in range(EC):
                            nc.tensor.matmul(
                                ps[:, :cw],
                                lhsT=w_sb[:, ec, pr * P:(pr + 1) * P],
                                rhs=xt[:, ec, c0:c0 + cw],
                                start=(ec == 0), stop=(ec == EC - 1))
                        nc.vector.tensor_scalar_add(
                            OUT[:, pr, c0:c0 + cw], ps[:, :cw],
                            b_sb[:, pr:pr + 1])

                    # pr0 K + pr0 Q (first 512 q-slice): needed by unit 0.
                    for c0, cw in kq_chunks:
                        kq_chunk(vps, "k", 0, c0, cw)
                    kq_chunk(vps, "q", 0, 0, 512)

            # ---------- filler quanta (PE work fed into the ACT-bound
            # attention stream; ordered so every chunk lands before its
            # consumer unit) ----------
            with tc.tile_pool(name=f"sp{_rep}", bufs=2, space="PSUM") as spp, \
                 tc.tile_pool(name=f"aps{_rep}", bufs=2, space="PSUM") as apsp, \
                 tc.tile_pool(name=f"kqp{_rep}", bufs=1, space="PSUM") as kqps, \
                 tc.tile_pool(name=f"ops{_rep}", bufs=1, space="PSUM") as ops:

                n_qh = s // 512      # 512-wide q units
                # group-0 fillers: K+first-q-slice of later prs, then the
                # qh=1 q-slices (deadline: before group 1).
                fillers = []
                for pr in range(1, FC):
                    for c0, cw in kq_chunks:
                        fillers.append(("k", pr, c0, cw))
                    fillers.append(("q", pr, 0, 512))
                if n_qh > 1:
                    for pr in range(FC):
                        fillers.append(("q", pr, 512, 512))

                o_cur = {}

                def oproj_quantum(qb, et, pool=None):
                    if et == 0:
                        o_cur[qb] = dout.tile([P, E], bf16, tag="o_sb",
                                              name="o_sb")
                    o = o_cur[qb]
                    ps = (pool or ops).tile([P, 512], f32, tag="ops",
                                            name="ps_o")
                    for fc in range(FC):
                        nc.tensor.matmul(
                            ps, lhsT=AC[:, fc, qb * P:(qb + 1) * P],
                            rhs=wo_sb[:, fc, et * 512:(et + 1) * 512],
                            start=(fc == 0), stop=(fc == FC - 1))
                    nc.vector.tensor_add(
                        o[:, et * 512:(et + 1) * 512], ps,
                        bo_bc[:, et * 512:(et + 1) * 512])
                    if et == E // 512 - 1:
                        nc.sync.dma_start(out_d[qb * P:(qb + 1) * P, :], o)
                        del o_cur[qb]

                def pop_filler(o_pool=None):
                    if fillers:
                        item = fillers.pop(0)
                        if item[0] in ("k", "q"):
                            kq_chunk(kqps, *item)
                        else:
                            oproj_quantum(item[1], item[2], o_pool)

                # ---------- attention units ----------
                # Both heads of the pr pair run over a 512-wide q slice per
                # unit.  Their K=64 score MMs write the two BANKS of one
                # [128,1024] psum tile (cols 0-511 / 512-1023) from PE row
                # halves 0-63 / 64-127 (tile_position auto-derived from the
                # operands' base partition), so the pair runs CONCURRENTLY
                # on distinct PE row tiles; the single exp covering both
                # heads frees both banks with one semaphore, preserving the
                # pairing for the next kb.
                def attention_unit(pr, qh, n_fill, pre_pops):
                    q0 = qh * 512
                    a_ps = [apsp.tile([D + 1, 512], f32, tag="aps",
                                      name=f"a_ps{j}") for j in range(2)]
                    es = {}
                    post = [3, 5, 7][:max(0, n_fill - pre_pops)]

                    def scores_pair(kb):
                        sp = spp.tile([P, QW], f32, tag="sp", name="sp")
                        for j in range(2):
                            base = j * 64
                            nc.tensor.matmul(
                                sp[:, j * 512:(j + 1) * 512],
                                lhsT=KT[base:base + 64, pr,
                                        kb * P:(kb + 1) * P],
                                rhs=QT[base:base + 64, pr, q0:q0 + 512],
                                start=True, stop=True)
                        e = epool.tile([P, QW], bf16, tag="e")
                        nc.scalar.activation(e, sp, EXP, scale=0.125)
                        es[kb] = e

                    def attnv(kb):
                        e = es.pop(kb)
                        for j in range(2):
                            h = 2 * pr + j
                            lv = vaug[kb][:, h * (D + 1):(h + 1) * (D + 1)]
                            nc.tensor.matmul(
                                a_ps[j], lhsT=lv,
                                rhs=e[:, j * 512:(j + 1) * 512],
                                start=(kb == 0), stop=(kb == n_kb - 1))

                    scores_pair(0)
                    for kb in range(1, n_kb):
                        scores_pair(kb)
                        if kb == 1:
                            for _ in range(pre_pops):
                                pop_filler()
                        attnv(kb - 1)
                        if kb in post:
                            pop_filler()
                    attnv(n_kb - 1)
                    for j in range(2):
                        rec = npool.tile([1, 512], f32, tag="rec",
                                         name=f"rec{j}")
                        nc.vector.reciprocal(rec, a_ps[j][D:D + 1, :])
                        rb = npool.tile([64, 512], f32, tag="rb",
                                        name=f"rb{j}")
                        nc.gpsimd.partition_broadcast(rb, rec)
                        nc.vector.tensor_mul(
                            AC[j * 64:(j + 1) * 64, pr, q0:q0 + 512],
                            a_ps[j][0:D, :], rb)

                # group 0 consumes the projection fillers; groups 1+ consume
                # their q-slice's out-proj quanta + q-projections needed two
                # groups ahead.
                unit = 0
                for qh in range(n_qh):
                    for pr in range(FC):
                        if unit < 4:
                            nf, pre = 4, 1
                        elif unit < 12:
                            nf, pre = 3, 2
                        else:
                            nf, pre = 2, 2
                        attention_unit(pr, qh, nf, pre)
                        unit += 1
                    o_new = [("o", qb, et)
                             for qb in range(qh * 4, (qh + 1) * 4)
                             for et in range(E // 512)]
                    q_next = ([("q", pr, (qh + 2) * 512, 512)
                               for pr in range(FC)]
                              if (qh + 2) * 512 < s else [])
                    mixed = []
                    for i, oq in enumerate(o_new):
                        mixed.append(oq)
                        if i % 2 == 1 and q_next:
                            mixed.append(q_next.pop(0))
                    mixed.extend(q_next)
                    fillers.extend(mixed)

            # ---------- tail: drain remaining out-proj quanta with a
            # deep psum pool (the attention pools are closed by now), so
            # consecutive quanta never serialize on the DVE bias-add.
            assert all(f[0] == "o" for f in fillers)
            with tc.tile_pool(name=f"otail{_rep}", bufs=4,
                              space="PSUM") as otail:
                while fillers:
                    pop_filler(o_pool=otail)

    nc.compile()
    _NC_CACHE[key] = nc
    return nc


def _slab_x(x):
    """[L, E] activation -> [P, EC, L] SBUF slab layout (pre-transposed)."""
    L = x.shape[0]
    return np.ascontiguousarray(x.T.reshape(EC, P, L).transpose(1, 0, 2))


def _slab_w(w):
    """[E, FL] weight -> [P, EC, FL]."""
    return np.ascontiguousarray(w.reshape(EC, P, FL).transpose(1, 0, 2))


def _slab_wo(w):
    """[FL, E] weight -> [P, FC, E]."""
    return np.ascontiguousarray(w.reshape(FC, P, E).transpose(1, 0, 2))


def make_in_maps(query, key, value, mask, Wq, bq, Wk, bk, Wv, bv, Wo, bo,
                 s=S, s_k=SK_PAD):
    """Shard full inputs into the 8 per-core input maps (bf16, pre-slabbed).

    key/value/mask rows are compacted per batch to the unmasked positions
    (masked rows contribute exactly 0 via the V-mask trick, so dropping
    them is exact) and padded with mask=0 rows."""
    import ml_dtypes
    bf16 = ml_dtypes.bfloat16
    query = np.asarray(query, np.float32).astype(bf16)
    key = np.asarray(key, np.float32).astype(bf16)
    value = np.asarray(value, np.float32).astype(bf16)
    Wq = np.asarray(Wq, np.float32).astype(bf16)
    Wk = np.asarray(Wk, np.float32).astype(bf16)
    Wv = np.asarray(Wv, np.float32).astype(bf16)
    Wo = np.asarray(Wo, np.float32).astype(bf16)
    bq = np.asarray(bq, np.float32)
    bk = np.asarray(bk, np.float32)
    bv = np.asarray(bv, np.float32)
    bo = np.asarray(bo, np.float32)
    maskf = np.asarray(mask).reshape(B, -1).astype(np.float32)  # (B, S)

    key_c = np.zeros((B, s_k, E), bf16)
    val_c = np.zeros((B, s_k, E), bf16)
    mask_c = np.zeros((B, s_k), np.float32)
    for b in range(B):
        idx = np.nonzero(maskf[b, :s])[0][:s_k]
        n = len(idx)
        key_c[b, :n] = key[b, idx]
        val_c[b, :n] = value[b, idx]
        mask_c[b, :n] = 1.0

    zeros_bo = np.zeros_like(bo)
    in_maps = []
    for c in range(NCORES):
        b, g = divmod(c, G)
        fs = slice(g * FL, (g + 1) * FL)
        in_maps.append({
            "xq": _slab_x(query[b, :s]),
            "xk": _slab_x(key_c[b]),
            "xv": _slab_x(val_c[b]),
            "wq": _slab_w(Wq[:, fs]),
            "wk": _slab_w(Wk[:, fs]),
            "wv": _slab_w(Wv[:, fs]),
            "wo": _slab_wo(Wo[fs, :]),
            "bq": np.ascontiguousarray(bq[fs].reshape(FC, P).T),
            "bk": np.ascontiguousarray(bk[fs].reshape(FC, P).T),
            "bv": np.ascontiguousarray(bv[fs]),
            "bo": bo if g == 0 else zeros_bo,
            "maskf": mask_c[b],
        })
    return in_maps


def gather_out(results):
    """Sum the per-core partial bf16 outputs into the full fp32 output."""
    out = np.zeros((B, S, E), np.float32)
    for c in range(NCORES):
        b, _ = divmod(c, G)
        out[b] += np.asarray(results[c]["out"], dtype=np.float32)
    return out


def kernel(query, key, value, mask, Wq, bq, Wk, bk, Wv, bv, Wo, bo):
    global LAST_RESULTS
    from concourse import bass_utils

    counts = np.asarray(mask).reshape(B, -1).sum(axis=1)
    s_k = SK_PAD if counts.max() <= SK_PAD else S
    nc = build_nc(S, s_k=s_k)
    in_maps = make_in_maps(query, key, value, mask,
                           Wq, bq, Wk, bk, Wv, bv, Wo, bo, s=S, s_k=s_k)
    trace = bool(int(os.environ.get("KTRACE", "0")))
    if trace:
        try:
            from antenv.axon_hooks import get_axon_ntff_profile_hook  # noqa: F401
        except ImportError:
            trace = False
    res = bass_utils.run_bass_kernel_spmd(
        nc, in_maps, core_ids=list(range(NCORES)), trace=trace)
    LAST_RESULTS = res
    return gather_out([res.results[c] for c in range(NCORES)])


# revision 34
# speedup vs baseline: 1.8272x; 1.8272x over previous
"""Self-contained 8-core Trainium2 Bass kernel for multi-head attention.

Problem: B=4, S=2048, E=1024, H=16, D=64 MHA with key-position mask.
Sharding: 8 cores = 4 batches x 2 head-groups (8 heads / 512 feats each).
Each core computes QKV projections for its (batch, head-group), attention,
and a partial output projection; the host sums the two partial outputs per
batch (bo folded into group-0 cores) in fp32.

Perf design (vs the v1 kernel):
  - Host pre-transposes + pre-slabs every tensor into the exact SBUF layout
    ([128, EC, S] etc.), so all device DMAs are full-rate plain copies --
    no on-device DMA transposes, and V-projection starts ~4us in.
  - The attention inner loop is software-pipelined per (pr, j, qt) unit:
    scores(kb+1) is emitted before attnV(kb), so the PE never sits waiting
    on the ACT exp; the ACT engine paces the loop at ~1.04us/kb.
  - All remaining projection work (K/Q proj of later head-pairs, out-proj)
    is chopped into ~1.7us matmul "quanta" and interleaved into the
    ACT-bound attention stream as PE filler.  The PE therefore runs
    near-continuously, which also keeps the HAM clock-gate at 2.4 GHz
    (idle-y kernels get throttled to 1.2 GHz).
  - PSUM budget (8 banks): scores double-buffer 2x[128,1024] (4 banks) +
    attnV accumulator [65,1024] (2) + proj filler [128,512] (1) +
    out-proj filler [128,512] (1).
  - mask folded into V: V_aug = [mask*V | mask] per head, so the softmax
    mask AND the denominator (row 64 of the attnV PSUM) are free; keys are
    host-compacted to the unmasked positions (exact), padded to s_k=1152.
  - exp has no max-subtraction (scores ~ N(0,1)); 1/sqrt(D) folded into
    the ACT scale field.  Output is bf16 (host accumulates in fp32).
"""

import os
import numpy as np

B = 4
S = 2048
E = 1024
H = 16
D = 64
G = 2                 # head groups (tensor parallel)
HL = H // G           # heads per core = 8
FL = HL * D           # local features = 512
P = 128
EC = E // P           # 8 e-chunks
FC = FL // P          # 4 local feature chunks
NCORES = 8
QW = 1024             # q tile width in attention

SK_PAD = 1152   # compacted key-dim (5.7 sigma above Binomial(2048, .5) mean)

_NC_CACHE = {}
LAST_RESULTS = None


def build_nc(s=S, s_k=SK_PAD, repeat=1, phases="ABCD"):
    """Build (and cache) the single-core Bass module, SPMD across 8 cores.

    repeat > 1 re-emits the whole computation N times inside one NEFF
    (timing aid: device time scales with repeat, launch overhead doesn't).
    """
    key = (s, s_k, repeat, phases)
    if key in _NC_CACHE:
        return _NC_CACHE[key]

    import concourse.mybir as mybir
    import concourse.tile as tile
    from concourse import bacc

    f32 = mybir.dt.float32
    bf16 = mybir.dt.bfloat16
    EXP = mybir.ActivationFunctionType.Exp

    n_kb = s_k // P       # key blocks
    n_qt = s // QW        # q units per (pr, j)
    n_qb = s // P         # out-proj q blocks
    kq_chunks = []        # (offset, width<=512) chunks covering s_k
    off = 0
    while off < s_k:
        w = min(512, s_k - off)
        kq_chunks.append((off, w))
        off += w
    # input DMA chunks for xk/xv: groups of 3 key-blocks (384 cols = 768B
    # contiguous per (p, ec) run -> full DMA rate, but fine-grained enough
    # that V-proj starts after the first chunk).
    x_chunks = []
    off = 0
    while off < s_k:
        w = min(3 * P, s_k - off)
        x_chunks.append((off, w))
        off += w
    # xv lands on the critical path to the first PE work: start with a
    # single key-block (half-rate 256B runs but tiny) then ramp up.
    v_chunks = [(0, P), (P, 2 * P)] + [(c0, cw) for c0, cw in x_chunks
                                       if c0 >= 3 * P]

    nc = bacc.Bacc("TRN2", target_bir_lowering=False, debug=False,
                   num_devices=NCORES)

    xq_d = nc.dram_tensor("xq", [P, EC, s], bf16, kind="ExternalInput").ap()
    xk_d = nc.dram_tensor("xk", [P, EC, s_k], bf16, kind="ExternalInput").ap()
    xv_d = nc.dram_tensor("xv", [P, EC, s_k], bf16, kind="ExternalInput").ap()
    wq_d = nc.dram_tensor("wq", [P, EC, FL], bf16, kind="ExternalInput").ap()
    wk_d = nc.dram_tensor("wk", [P, EC, FL], bf16, kind="ExternalInput").ap()
    wv_d = nc.dram_tensor("wv", [P, EC, FL], bf16, kind="ExternalInput").ap()
    wo_d = nc.dram_tensor("wo", [P, FC, E], bf16, kind="ExternalInput").ap()
    bq_d = nc.dram_tensor("bq", [P, FC], f32, kind="ExternalInput").ap()
    bk_d = nc.dram_tensor("bk", [P, FC], f32, kind="ExternalInput").ap()
    bv_d = nc.dram_tensor("bv", [FL], f32, kind="ExternalInput").ap()
    bo_d = nc.dram_tensor("bo", [E], f32, kind="ExternalInput").ap()
    mask_d = nc.dram_tensor("maskf", [s_k], f32, kind="ExternalInput").ap()
    out_d = nc.dram_tensor("out", [s, E], bf16, kind="ExternalOutput").ap()

    with tile.TileContext(nc) as tc:
      for _rep in range(repeat):
        with tc.tile_pool(name=f"consts{_rep}", bufs=1) as consts, \
             tc.tile_pool(name=f"persist{_rep}", bufs=1) as persist, \
             tc.tile_pool(name=f"xtkq{_rep}", bufs=1) as xtkq, \
             tc.tile_pool(name=f"ep{_rep}", bufs=4) as epool, \
             tc.tile_pool(name=f"np{_rep}", bufs=2) as npool, \
             tc.tile_pool(name=f"dout{_rep}", bufs=3) as dout:

            # ---------- constant / persistent SBUF ----------
            bv_row = consts.tile([1, FL], f32, tag="bv_row")
            maskc = consts.tile([P, n_kb], f32, tag="maskc")
            wv_sb = consts.tile([P, EC, FL], bf16, tag="wv")
            wk_sb = consts.tile([P, EC, FL], bf16, tag="wk")
            wq_sb = consts.tile([P, EC, FL], bf16, tag="wq")
            bk_sb = consts.tile([P, FC], f32, tag="bk")
            bq_sb = consts.tile([P, FC], f32, tag="bq")
            wo_sb = consts.tile([P, FC, E], bf16, tag="wo")
            bo_row = consts.tile([1, E], f32, tag="bo_row")

            xt_k = xtkq.tile([P, EC, s_k], bf16, tag="xtk")
            xt_q = xtkq.tile([P, EC, s], bf16, tag="xtq")

            KT = persist.tile([P, FC, s_k], bf16, tag="KT")
            QT = persist.tile([P, FC, s], bf16, tag="QT")
            AC = persist.tile([P, FC, s], bf16, tag="AC")     # attn_cat^T
            vaug = [persist.tile([P, HL * (D + 1)], bf16, tag=f"vaug{kb}",
                                 name=f"vaug{_rep}_{kb}")
                    for kb in range(n_kb)]

            # ---------- input DMAs ----------
            # All on the sync (SP) ring, in strict priority order (the V
            # path first so V-proj starts ~6us in).  Keeping the ACT ring
            # DMA-free leaves the Activation sequencer 100% for exps.
            with tc.tile_pool(name=f"xtv{_rep}", bufs=1) as xtvp:
                xt_v = xtvp.tile([P, EC, s_k], bf16, tag="xtv")
                nc.sync.dma_start(wv_sb[:, 0:EC // 2, :],
                                  wv_d[:, 0:EC // 2, :])
                c0f, cwf = v_chunks[0]
                nc.sync.dma_start(xt_v[:, :, c0f:c0f + cwf],
                                  xv_d[:, :, c0f:c0f + cwf])
                nc.sync.dma_start(wv_sb[:, EC // 2:, :],
                                  wv_d[:, EC // 2:, :])
                for c0, cw in v_chunks[1:]:
                    nc.sync.dma_start(xt_v[:, :, c0:c0 + cw],
                                      xv_d[:, :, c0:c0 + cw])
                nc.sync.dma_start(bv_row, bv_d[None, :])
                nc.sync.dma_start(maskc,
                                  mask_d.rearrange("(c p) -> p c", p=P))
                nc.sync.dma_start(wk_sb, wk_d)
                nc.sync.dma_start(bk_sb, bk_d)
                for c0, cw in x_chunks:
                    nc.sync.dma_start(xt_k[:, :, c0:c0 + cw],
                                      xk_d[:, :, c0:c0 + cw])
                nc.sync.dma_start(wq_sb, wq_d)
                nc.sync.dma_start(bq_sb, bq_d)
                for c0 in range(0, s, QW):
                    nc.sync.dma_start(xt_q[:, :, c0:c0 + QW],
                                      xq_d[:, :, c0:c0 + QW])
                nc.sync.dma_start(wo_sb, wo_d)
                nc.sync.dma_start(bo_row, bo_d[None, :])

                # broadcast rows (gpsimd)
                bv_bc = consts.tile([P, FL], f32, tag="bv_bc")
                nc.gpsimd.partition_broadcast(bv_bc, bv_row)
                bo_bc = consts.tile([P, E], f32, tag="bo_bc")
                nc.gpsimd.partition_broadcast(bo_bc, bo_row)
                maskc_bf = consts.tile([P, n_kb], bf16, tag="maskc_bf")
                nc.vector.tensor_copy(maskc_bf, maskc)

                # ---------- head phase: V projection + pr0 K/Q ----------
                with tc.tile_pool(name=f"vps{_rep}", bufs=2,
                                  space="PSUM") as vps:
                    for kb in range(n_kb):
                        ps = vps.tile([P, FL], f32, tag="vps", name="ps_v")
                        for ec in range(EC):
                            nc.tensor.matmul(
                                ps, lhsT=xt_v[:, ec, kb * P:(kb + 1) * P],
                                rhs=wv_sb[:, ec, :],
                                start=(ec == 0), stop=(ec == EC - 1))
                        vrow = vaug[kb].rearrange("p (h c) -> p h c", c=D + 1)
                        nc.vector.tensor_add(
                            vrow[:, :, 0:D],
                            ps.rearrange("p (h d) -> p h d", d=D),
                            bv_bc.rearrange("p (h d) -> p h d", d=D))
                        nc.vector.tensor_scalar_mul(
                            vrow[:, :, 0:D], vrow[:, :, 0:D],
                            maskc[:, kb:kb + 1])
                        nc.vector.tensor_copy(
                            vrow[:, :, D:D + 1],
                            maskc_bf[:, kb:kb + 1, None].to_broadcast(
                                [P, HL, 1]))

                    def kq_chunk(psum_pool, kind, pr, c0, cw):
                        w_sb, b_sb, xt, OUT = (
                            (wk_sb, bk_sb, xt_k, KT) if kind == "k"
                            else (wq_sb, bq_sb, xt_q, QT))
                        ps = psum_pool.tile([P, 512], f32, tag="kq",
                                            name=f"ps_{kind}")
                        for ec in range(EC):
                            nc.tensor.matmul(
                                ps[:, :cw],
                                lhsT=w_sb[:, ec, pr * P:(pr + 1) * P],
                                rhs=xt[:, ec, c0:c0 + cw],
                                start=(ec == 0), stop=(ec == EC - 1))
                        nc.vector.tensor_scalar_add(
                            OUT[:, pr, c0:c0 + cw], ps[:, :cw],
                            b_sb[:, pr:pr + 1])

                    # pr0 K + pr0 Q (first qt half): needed before unit 0.
                    for c0, cw in kq_chunks:
                        kq_chunk(vps, "k", 0, c0, cw)
                    for c0 in range(0, QW, 512):
                        kq_chunk(vps, "q", 0, c0, 512)

            # ---------- filler quanta (PE work fed into the ACT-bound
            # attention stream; ordered so every chunk lands before its
            # consumer unit) ----------
            with tc.tile_pool(name=f"sp{_rep}", bufs=2, space="PSUM") as spp, \
                 tc.tile_pool(name=f"aps{_rep}", bufs=1, space="PSUM") as apsp, \
                 tc.tile_pool(name=f"kqp{_rep}", bufs=1, space="PSUM") as kqps, \
                 tc.tile_pool(name=f"ops{_rep}", bufs=1, space="PSUM") as ops:

                fillers = []
                for pr in range(1, FC):
                    for c0, cw in kq_chunks:
                        fillers.append(("k", pr, c0, cw))
                    for c0 in range(0, QW, 512):
                        fillers.append(("q", pr, c0, 512))
                for pr in range(2):
                    for c0 in range(QW, s, 512):
                        fillers.append(("q", pr, c0, 512))
                # Q-proj qt1 chunks for pr2/pr3 are deadline-late; they are
                # interleaved into the qt1 units (below) as bigger filler
                # quanta that cover the unit-boundary a_ps stall.
                late_q = [("q", pr, c0, 512)
                          for pr in range(2, FC)
                          for c0 in range(QW, s, 512)]

                o_cur = {}

                def oproj_quantum(qb, et, pool=None):
                    if et == 0:
                        o_cur[qb] = dout.tile([P, E], bf16, tag="o_sb",
                                              name="o_sb")
                    o = o_cur[qb]
                    ps = (pool or ops).tile([P, 512], f32, tag="ops",
                                            name="ps_o")
                    for fc in range(FC):
                        nc.tensor.matmul(
                            ps, lhsT=AC[:, fc, qb * P:(qb + 1) * P],
                            rhs=wo_sb[:, fc, et * 512:(et + 1) * 512],
                            start=(fc == 0), stop=(fc == FC - 1))
                    nc.vector.tensor_add(
                        o[:, et * 512:(et + 1) * 512], ps,
                        bo_bc[:, et * 512:(et + 1) * 512])
                    if et == E // 512 - 1:
                        nc.sync.dma_start(out_d[qb * P:(qb + 1) * P, :], o)
                        del o_cur[qb]

                def pop_filler(o_pool=None):
                    if fillers:
                        item = fillers.pop(0)
                        if item[0] in ("k", "q"):
                            kq_chunk(kqps, *item)
                        else:
                            oproj_quantum(item[1], item[2], o_pool)

                # ---------- attention units ----------
                def attention_unit(pr, j, qt, n_fill):
                    base = j * 64
                    h = 2 * pr + j
                    q0 = qt * QW
                    rhs_q = QT[base:base + 64, pr, q0:q0 + QW]
                    a_ps = apsp.tile([D + 1, QW], f32, tag="aps",
                                     name="a_ps")
                    es = {}
                    # fillers right before attnv(0) (it waits on the
                    # previous unit's normalize freeing a_ps): qt1 units
                    # (small out-proj quanta) put both there; qt0 units
                    # (1.7us proj quanta) put one there, rest spread.
                    if n_fill <= 2:
                        pre_pops, post_slots = n_fill, set()
                    else:
                        pre_pops = 1
                        post_slots = {3 + i * 3 for i in range(n_fill - 1)}

                    def scores(kb):
                        sp = spp.tile([P, QW], f32, tag="sp", name="sp")
                        lhsT = KT[base:base + 64, pr, kb * P:(kb + 1) * P]
                        for hf in range(QW // 512):
                            nc.tensor.matmul(
                                sp[:, hf * 512:(hf + 1) * 512], lhsT=lhsT,
                                rhs=rhs_q[:, hf * 512:(hf + 1) * 512],
                                start=True, stop=True)
                        e = epool.tile([P, QW], bf16, tag="e")
                        nc.scalar.activation(e, sp, EXP, scale=0.125)
                        es[kb] = e

                    def attnv(kb):
                        e = es.pop(kb)
                        lv = vaug[kb][:, h * (D + 1):(h + 1) * (D + 1)]
                        for hf in range(QW // 512):
                            nc.tensor.matmul(
                                a_ps[:, hf * 512:(hf + 1) * 512], lhsT=lv,
                                rhs=e[:, hf * 512:(hf + 1) * 512],
                                start=(kb == 0), stop=(kb == n_kb - 1))

                    scores(0)
                    for kb in range(1, n_kb):
                        scores(kb)
                        if kb == 1:
                            for _ in range(pre_pops):
                                pop_filler()
                        attnv(kb - 1)
                        if kb in post_slots:
                            pop_filler()
                    attnv(n_kb - 1)
                    # normalize: AC[j-half, pr, q0:q0+QW] = a_ps[0:D] / denom
                    rec = npool.tile([1, QW], f32, tag="rec")
                    nc.vector.reciprocal(rec, a_ps[D:D + 1, :])
                    rb = npool.tile([64, QW], f32, tag="rb")
                    nc.gpsimd.partition_broadcast(rb, rec)
                    nc.vector.tensor_mul(
                        AC[j * 64:(j + 1) * 64, pr, q0:q0 + QW],
                        a_ps[0:D, :], rb)

                # per-unit filler pop counts, matched to the supply/deadline
                # schedule derived in the module docstring design.
                NFILL = [3, 3, 3, 3, 3, 2, 1, 1, 3, 3, 3, 3, 3, 3, 1, 1]
                unit = 0
                for qt in range(n_qt):
                    for pr in range(FC):
                        for j in range(2):
                            nf = NFILL[unit] if unit < len(NFILL) else 2
                            attention_unit(pr, j, qt, nf)
                            unit += 1
                    # queue this qt's out-proj quanta (legal now that all
                    # its units' AC columns are written)
                    o_new = [("o", qb, et)
                             for qb in range(qt * QW // P, (qt + 1) * QW // P)
                             for et in range(E // 512)]
                    if qt == 0:
                        mixed = []
                        for i, oq in enumerate(o_new):
                            mixed.append(oq)
                            if i % 2 == 1 and late_q:
                                mixed.append(late_q.pop(0))
                        mixed.extend(late_q)
                        late_q = []
                        fillers.extend(mixed)
                    else:
                        fillers.extend(o_new)

            # ---------- tail: drain remaining out-proj quanta with a
            # deep psum pool (the attention pools are closed by now), so
            # consecutive quanta never serialize on the DVE bias-add.
            assert all(f[0] == "o" for f in fillers)
            with tc.tile_pool(name=f"otail{_rep}", bufs=4,
                              space="PSUM") as otail:
                while fillers:
                    pop_filler(o_pool=otail)

    nc.compile()
    _NC_CACHE[key] = nc
    return nc


def _slab_x(x):
    """[L, E] activation -> [P, EC, L] SBUF slab layout (pre-transposed)."""
    L = x.shape[0]
    return np.ascontiguousarray(x.T.reshape(EC, P, L).transpose(1, 0, 2))


def _slab_w(w):
    """[E, FL] weight -> [P, EC, FL]."""
    return np.ascontiguousarray(w.reshape(EC, P, FL).transpose(1, 0, 2))


def _slab_wo(w):
    """[FL, E] weight -> [P, FC, E]."""
    return np.ascontiguousarray(w.reshape(FC, P, E).transpose(1, 0, 2))


def make_in_maps(query, key, value, mask, Wq, bq, Wk, bk, Wv, bv, Wo, bo,
                 s=S, s_k=SK_PAD):
    """Shard full inputs into the 8 per-core input maps (bf16, pre-slabbed).

    key/value/mask rows are compacted per batch to the unmasked positions
    (masked rows contribute exactly 0 via the V-mask trick, so dropping
    them is exact) and padded with mask=0 rows."""
    import ml_dtypes
    bf16 = ml_dtypes.bfloat16
    query = np.asarray(query, np.float32).astype(bf16)
    key = np.asarray(key, np.float32).astype(bf16)
    value = np.asarray(value, np.float32).astype(bf16)
    Wq = np.asarray(Wq, np.float32).astype(bf16)
    Wk = np.asarray(Wk, np.float32).astype(bf16)
    Wv = np.asarray(Wv, np.float32).astype(bf16)
    Wo = np.asarray(Wo, np.float32).astype(bf16)
    bq = np.asarray(bq, np.float32)
    bk = np.asarray(bk, np.float32)
    bv = np.asarray(bv, np.float32)
    bo = np.asarray(bo, np.float32)
    maskf = np.asarray(mask).reshape(B, -1).astype(np.float32)  # (B, S)

    key_c = np.zeros((B, s_k, E), bf16)
    val_c = np.zeros((B, s_k, E), bf16)
    mask_c = np.zeros((B, s_k), np.float32)
    for b in range(B):
        idx = np.nonzero(maskf[b, :s])[0][:s_k]
        n = len(idx)
        key_c[b, :n] = key[b, idx]
        val_c[b, :n] = value[b, idx]
        mask_c[b, :n] = 1.0

    zeros_bo = np.zeros_like(bo)
    in_maps = []
    for c in range(NCORES):
        b, g = divmod(c, G)
        fs = slice(g * FL, (g + 1) * FL)
        in_maps.append({
            "xq": _slab_x(query[b, :s]),
            "xk": _slab_x(key_c[b]),
            "xv": _slab_x(val_c[b]),
            "wq": _slab_w(Wq[:, fs]),
            "wk": _slab_w(Wk[:, fs]),
            "wv": _slab_w(Wv[:, fs]),
            "wo": _slab_wo(Wo[fs, :]),
            "bq": np.ascontiguousarray(bq[fs].reshape(FC, P).T),
            "bk": np.ascontiguousarray(bk[fs].reshape(FC, P).T),
            "bv": np.ascontiguousarray(bv[fs]),
            "bo": bo if g == 0 else zeros_bo,
            "maskf": mask_c[b],
        })
    return in_maps


def gather_out(results):
    """Sum the per-core partial bf16 outputs into the full fp32 output."""
    out = np.zeros((B, S, E), np.float32)
    for c in range(NCORES):
        b, _ = divmod(c, G)
        out[b] += np.asarray(results[c]["out"], dtype=np.float32)
    return out


def kernel(query, key, value, mask, Wq, bq, Wk, bk, Wv, bv, Wo, bo):
    global LAST_RESULTS
    from concourse import bass_utils

    counts = np.asarray(mask).reshape(B, -1).sum(axis=1)
    s_k = SK_PAD if counts.max() <= SK_PAD else S
    nc = build_nc(S, s_k=s_k)
    in_maps = make_in_maps(query, key, value, mask,
                           Wq, bq, Wk, bk, Wv, bv, Wo, bo, s=S, s_k=s_k)
    trace = bool(int(os.environ.get("KTRACE", "0")))
    if trace:
        try:
            from antenv.axon_hooks import get_axon_ntff_profile_hook  # noqa: F401
        except ImportError:
            trace = False
    res = bass_utils.run_bass_kernel_spmd(
        nc, in_maps, core_ids=list(range(NCORES)), trace=trace)
    LAST_RESULTS = res
    return gather_out([res.results[c] for c in range(NCORES)])
